# revision 7
# baseline (speedup 1.0000x reference)
"""CompressionHead kernel for Trainium2 (8 NeuronCores, Bass/Tile).

Reference computes:
    u          = h / max(||h||_2, eps)              (row-normalize, dim=-1)
    sim        = einsum('bid,bjd->bij', u, u)       (B,S,S) batched GEMM
    conc       = (sum(sim) - trace(sim)) / (B*S*(S-1))
    lambda_t   = sigmoid(alpha * (conc - beta))
    returns (lambda_t, conc)

Key identity: sum_{i,j} u_i . u_j = || sum_i u_i ||^2, so the O(B*S^2*D)
GEMM collapses to an O(B*S*D) normalize-and-reduce:
    sum(sim)   = sum_b || s_b ||^2,   s_b = sum_i u_{b,i}
    trace(sim) = sum_{b,i} u_{b,i} . u_{b,i}   (~= B*S)

Sharding: flatten (B,S) -> 16384 rows; each of the 8 cores takes a
contiguous 2048-row block (2 cores per batch; blocks never straddle a
batch). Per core, per [128, 2048] row-tile:
  - DVE tensor_tensor_reduce: ss[p] = sum_d x[p,d]^2
  - ACT sqrt + DVE reciprocal: inv[p] = 1/||x_p||
  - PE matmul (lhsT=inv [128,1], rhs=x): psum[1,:] += sum_p inv[p]*x[p,:]
    accumulated across the 16 row-tiles (u is never materialized)
  - diag column: ss * inv^2  (per-row u.u, matches reference to f32 noise)
Host combines the tiny per-core outputs in float64.
"""

import numpy as np

B, S, D = 4, 4096, 2048
N_CORES = 8
ROWS_PER_CORE = (B * S) // N_CORES  # 2048
P = 128
N_TILES = ROWS_PER_CORE // P  # 16
N_CHUNK = 512  # fp32 matmul moving-operand / PSUM-bank limit
N_CHUNKS = D // N_CHUNK  # 4

_CACHE = {}


def _build_nc():
    import concourse.tile as tile
    from concourse import bacc, mybir

    F32 = mybir.dt.float32
    nc = bacc.Bacc(None, target_bir_lowering=False, debug=True)
    x = nc.dram_tensor("x", [ROWS_PER_CORE, D], F32, kind="ExternalInput")
    s_out = nc.dram_tensor("s_out", [1, D], F32, kind="ExternalOutput")
    d_out = nc.dram_tensor("d_out", [P, N_TILES], F32, kind="ExternalOutput")

    with tile.TileContext(nc) as tc:
        with (
            tc.tile_pool(name="xp", bufs=4) as xp,
            tc.tile_pool(name="scratch", bufs=2) as scratch,
            tc.tile_pool(name="small", bufs=4) as small,
            tc.tile_pool(name="psum", bufs=1, space="PSUM") as pp,
            tc.tile_pool(name="outp", bufs=1) as outp,
        ):
            psums = [
                pp.tile([1, N_CHUNK], F32, name=f"ps{k}", tag=f"ps{k}")
                for k in range(N_CHUNKS)
            ]
            d_sb = outp.tile([P, N_TILES], F32, name="d_sb")
            s_sb = outp.tile([1, D], F32, name="s_sb")
            # Constant column of ones: stationary matmul operand. Written by
            # DVE so every matmul's deps (ones, u) live on one engine — the
            # TRN2 LDWEIGHTS ISA struct only fits a single sync wait.
            ones = outp.tile([P, 1], F32, name="ones")
            nc.vector.memset(ones[:], 1.0)

            for t in range(N_TILES):
                xt = xp.tile([P, D], F32, name="xt", tag="xt")
                nc.sync.dma_start(out=xt[:], in_=x[t * P : (t + 1) * P, :])

                # ACT: ss[p] = sum_d xt[p,d]^2 (sq is a throwaway scratch)
                sq = scratch.tile([P, D], F32, name="sq", tag="sq")
                ss = small.tile([P, 1], F32, name="ss", tag="ss")
                nc.scalar.activation(
                    sq[:],
                    xt[:],
                    mybir.ActivationFunctionType.Square,
                    accum_out=ss[:],
                )
                nrm = small.tile([P, 1], F32, name="nrm", tag="nrm")
                nc.scalar.sqrt(nrm[:], ss[:])
                inv = small.tile([P, 1], F32, name="inv", tag="inv")
                nc.vector.reciprocal(inv[:], nrm[:])

                # DVE: u = xt * inv  (the normalized rows, as in reference)
                u = scratch.tile([P, D], F32, name="u", tag="u")
                nc.vector.tensor_scalar_mul(u[:], xt[:], inv[:])

                # diag contribution of each row: ss * inv^2 == u . u
                ssi = small.tile([P, 1], F32, name="ssi", tag="ssi")
                nc.vector.tensor_mul(ssi[:], ss[:], inv[:])
                nc.vector.tensor_mul(d_sb[:, t : t + 1], ssi[:], inv[:])

                for k in range(N_CHUNKS):
                    nc.tensor.matmul(
                        psums[k][:],
                        lhsT=ones[:],
                        rhs=u[:, k * N_CHUNK : (k + 1) * N_CHUNK],
                        start=(t == 0),
                        stop=(t == N_TILES - 1),
                    )

            for k in range(N_CHUNKS):
                nc.vector.tensor_copy(
                    s_sb[:, k * N_CHUNK : (k + 1) * N_CHUNK], psums[k][:]
                )
            nc.sync.dma_start(out=s_out[:], in_=s_sb[:])
            nc.sync.dma_start(out=d_out[:], in_=d_sb[:])

    # Full bacc lowering: splits multi-sem waits into event semaphores,
    # moves matmul waits onto LDWEIGHTS, populates extended-inst ISA bytes.
    # Raw Bass skips all of this and walrus codegen rejects the result.
    nc.compile()
    return nc


def get_nc():
    if "nc" not in _CACHE:
        _CACHE["nc"] = _build_nc()
    return _CACHE["nc"]


def make_in_maps(h):
    flat = np.ascontiguousarray(np.asarray(h, dtype=np.float32)).reshape(B * S, D)
    return [
        {"x": flat[c * ROWS_PER_CORE : (c + 1) * ROWS_PER_CORE]}
        for c in range(N_CORES)
    ]


def finish(results, alpha, beta):
    """Combine per-core partial outputs (host, float64)."""
    s_parts = np.stack([np.asarray(r["s_out"][0], dtype=np.float64) for r in results])
    diag = float(sum(np.asarray(r["d_out"], dtype=np.float64).sum() for r in results))
    cores_per_batch = N_CORES // B
    s_b = s_parts.reshape(B, cores_per_batch, D).sum(axis=1)  # (B, D)
    sum_sim = float((s_b * s_b).sum())
    denom = float(B) * S * (S - 1)
    conc = (sum_sim - diag) / denom
    lam = 1.0 / (1.0 + np.exp(-(float(alpha) * (conc - float(beta)))))
    return (
        np.asarray(lam, dtype=np.float32),
        np.asarray(conc, dtype=np.float32),
    )


def kernel(h, alpha, beta):
    from concourse.bass_utils import run_bass_kernel_spmd

    nc = get_nc()
    in_maps = make_in_maps(h)
    results = run_bass_kernel_spmd(nc, in_maps, core_ids=list(range(N_CORES))).results
    return finish(results, alpha, beta)


# revision 15
# speedup vs baseline: 1.3208x; 1.3208x over previous
"""CompressionHead kernel for Trainium2 (8 NeuronCores, Bass/Tile).

Reference computes:
    u          = h / max(||h||_2, eps)              (row-normalize, dim=-1)
    sim        = einsum('bid,bjd->bij', u, u)       (B,S,S) batched GEMM
    conc       = (sum(sim) - trace(sim)) / (B*S*(S-1))
    lambda_t   = sigmoid(alpha * (conc - beta))
    returns (lambda_t, conc)

Key identity: sum_{i,j} u_i . u_j = || sum_i u_i ||^2, so the O(B*S^2*D)
GEMM collapses to an O(B*S*D) normalize-and-reduce:
    sum(sim)   = sum_b || s_b ||^2,   s_b = sum_i u_{b,i}
    trace(sim) = sum_{b,i} u_{b,i} . u_{b,i}   (~= B*S)

Sharding: flatten (B,S) -> 16384 rows; each of the 8 cores takes a
contiguous 2048-row block (2 cores per batch; blocks never straddle a
batch). Per core, per [128, 2048] row-tile:
  - row sum-of-squares ss split between ACT (Square+accum, cols 0..1023)
    and DVE (tensor_tensor_reduce, cols 1024..2047) so neither engine
    exceeds the DMA-bound tile time
  - ACT sqrt + DVE reciprocal: inv[p] = 1/||x_p||
  - PE matmul psum[1,:] += inv.T @ x accumulated over the 16 row-tiles
    (u never materialized; inv is folded into the MAC). Operands are
    bitcast to float32r: full-rate PE at slightly relaxed precision —
    conc error stays ~1e-4 (validated against the f32 reference).
  - diag column: ss * inv^2  (per-row u.u, matches reference to f32 noise)
Host combines the tiny per-core outputs in float64.
"""

import numpy as np

B, S, D = 4, 4096, 2048
N_CORES = 8
ROWS_PER_CORE = (B * S) // N_CORES  # 2048
P = 128
N_TILES = ROWS_PER_CORE // P  # 16
N_CHUNK = 512  # PSUM-bank / fp32 moving-operand limit per matmul
N_CHUNKS = D // N_CHUNK  # 4

MM_F32R = True  # PE matmul in float32r (full rate vs 4 cyc/row for f32)

_CACHE = {}


def _build_nc():
    import concourse.tile as tile
    from concourse import bacc, mybir

    F32 = mybir.dt.float32
    F32R = mybir.dt.float32r
    nc = bacc.Bacc(None, target_bir_lowering=False, debug=True)
    x = nc.dram_tensor("x", [ROWS_PER_CORE, D], F32, kind="ExternalInput")
    s_out = nc.dram_tensor("s_out", [1, D], F32, kind="ExternalOutput")
    d_out = nc.dram_tensor("d_out", [P, N_TILES], F32, kind="ExternalOutput")

    with tile.TileContext(nc) as tc:
        with (
            tc.tile_pool(name="xp", bufs=4) as xp,
            tc.tile_pool(name="scratch", bufs=2) as scratch,
            tc.tile_pool(name="small", bufs=4) as small,
            tc.tile_pool(name="psum", bufs=1, space="PSUM") as pp,
            tc.tile_pool(name="outp", bufs=1) as outp,
        ):
            psums = [
                pp.tile([1, N_CHUNK], F32, name=f"ps{k}", tag=f"ps{k}")
                for k in range(N_CHUNKS)
            ]
            d_sb = outp.tile([P, N_TILES], F32, name="d_sb")
            s_sb = outp.tile([1, D], F32, name="s_sb")

            H = D // 2
            for t in range(N_TILES):
                # xt typed f32r so the BIR verifier accepts it as an f32r
                # matmul operand (same 4-byte payload as f32); non-matmul
                # consumers read it bitcast back to f32.
                xt_dt = F32R if MM_F32R else F32
                xt = xp.tile([P, D], xt_dt, name="xt", tag="xt")
                src = x[t * P : (t + 1) * P, :]
                if MM_F32R:
                    src = src.bitcast(F32R)
                nc.sync.dma_start(out=xt[:], in_=src)
                xtf = xt[:].bitcast(F32) if MM_F32R else xt[:]

                # ss[p] = sum_d xt[p,d]^2, halves on ACT and DVE in parallel
                # (sqa/sqb are throwaway full-width outputs the ISA requires)
                sqa = scratch.tile([P, H], F32, name="sqa", tag="sqa")
                ssa = small.tile([P, 1], F32, name="ssa", tag="ssa")
                nc.scalar.activation(
                    sqa[:],
                    xtf[:, :H],
                    mybir.ActivationFunctionType.Square,
                    accum_out=ssa[:],
                )
                # DVE half: tensor_tensor_reduce would fuse these two, but
                # that extended op crashes the NEFF at runtime on this stack
                # (its DVE ucode table isn't delivered) — use plain ops.
                sqb = scratch.tile([P, H], F32, name="sqb", tag="sqb")
                nc.vector.tensor_mul(sqb[:], xtf[:, H:], xtf[:, H:])
                ssb = small.tile([P, 1], F32, name="ssb", tag="ssb")
                nc.vector.tensor_reduce(
                    ssb[:],
                    sqb[:],
                    axis=mybir.AxisListType.X,
                    op=mybir.AluOpType.add,
                )
                ss = small.tile([P, 1], F32, name="ss", tag="ss")
                nc.vector.tensor_add(ss[:], ssa[:], ssb[:])

                nrm = small.tile([P, 1], F32, name="nrm", tag="nrm")
                nc.scalar.sqrt(nrm[:], ss[:])
                inv = small.tile([P, 1], xt_dt, name="inv", tag="inv")
                if MM_F32R:
                    with nc.allow_low_precision(reason="f32r keeps f32 width"):
                        nc.vector.reciprocal(inv[:], nrm[:])
                else:
                    nc.vector.reciprocal(inv[:], nrm[:])
                invf = inv[:].bitcast(F32) if MM_F32R else inv[:]

                # diag contribution of each row: ss * inv^2 == u . u
                ssi = small.tile([P, 1], F32, name="ssi", tag="ssi")
                nc.vector.tensor_mul(ssi[:], ss[:], invf)
                nc.vector.tensor_mul(d_sb[:, t : t + 1], ssi[:], invf)

                for k in range(N_CHUNKS):
                    nc.tensor.matmul(
                        psums[k][:],
                        lhsT=inv[:],
                        rhs=xt[:, k * N_CHUNK : (k + 1) * N_CHUNK],
                        start=(t == 0),
                        stop=(t == N_TILES - 1),
                    )

            for k in range(N_CHUNKS):
                nc.vector.tensor_copy(
                    s_sb[:, k * N_CHUNK : (k + 1) * N_CHUNK], psums[k][:]
                )
            nc.sync.dma_start(out=s_out[:], in_=s_sb[:])
            nc.sync.dma_start(out=d_out[:], in_=d_sb[:])

    # Full bacc lowering: splits multi-sem waits into event semaphores,
    # moves matmul waits onto LDWEIGHTS, populates extended-inst ISA bytes.
    # Raw Bass skips all of this and walrus codegen rejects the result.
    nc.compile()
    return nc


def get_nc():
    if "nc" not in _CACHE:
        _CACHE["nc"] = _build_nc()
    return _CACHE["nc"]


def make_in_maps(h):
    flat = np.ascontiguousarray(np.asarray(h, dtype=np.float32)).reshape(B * S, D)
    return [
        {"x": flat[c * ROWS_PER_CORE : (c + 1) * ROWS_PER_CORE]}
        for c in range(N_CORES)
    ]


def finish(results, alpha, beta):
    """Combine per-core partial outputs (host, float64)."""
    s_parts = np.stack([np.asarray(r["s_out"][0], dtype=np.float64) for r in results])
    diag = float(sum(np.asarray(r["d_out"], dtype=np.float64).sum() for r in results))
    cores_per_batch = N_CORES // B
    s_b = s_parts.reshape(B, cores_per_batch, D).sum(axis=1)  # (B, D)
    sum_sim = float((s_b * s_b).sum())
    denom = float(B) * S * (S - 1)
    conc = (sum_sim - diag) / denom
    lam = 1.0 / (1.0 + np.exp(-(float(alpha) * (conc - float(beta)))))
    return (
        np.asarray(lam, dtype=np.float32),
        np.asarray(conc, dtype=np.float32),
    )


def kernel(h, alpha, beta):
    from concourse.bass_utils import run_bass_kernel_spmd

    nc = get_nc()
    in_maps = make_in_maps(h)
    results = run_bass_kernel_spmd(nc, in_maps, core_ids=list(range(N_CORES))).results
    return finish(results, alpha, beta)
